# revision 37
# baseline (speedup 1.0000x reference)
"""Trainium2 Bass kernel for nn_MultiHeadAttention_7584912245188.

Reference computes (no softmax!):
    qkv = x @ Wqkv + bqkv ; split q,k,v ; per head: y = (q k^T / sqrt(D)) v
    out = y @ Wff + bff

No softmax => attention is linear and reassociates: (Q K^T) V = Q (K^T V).
With X_aug = [X | 1] ([N, 97]) and G = X_aug^T X_aug ([97, 97]) the module
collapses to out = X_aug @ Wfin computed on device as:
    V = G @ Wk_aug                                  [97, 96]  (1 matmul)
    Call_h = V_h^T @ Wvff_h                         [16, 96]  (6 matmuls,
             head blocks along the free dim; Wvff_h = D^-0.5 Wv_aug_h Wff_h)
    Wfin = sum_h Wq_aug_h @ Call_h + e_last bff^T   [97, 96]  (7-matmul group)
    out rows {8p+j} = X_chunk @ Wfin                (8 matmuls via transposed X)
O(N*E^2) instead of O(N^2*D).

Sharding (8 cores): core c -> (batch b = c//2, half h = c%2). Each core
computes the full-batch Gram redundantly (cheaper than a collective) and
writes its own half of the rows.

Schedule (fixed DMA costs dominate: HWDGE issue ~630 + 650 DGE delay, SWDGE
prep ~1040 + 650, 900ns completion sem, shared ~360GB/s DMA engines):
  - near half rides the first sync-HWDGE slot in fp16 (Gram + transposes);
    the far half is QUANTIZED TO FP8 (it only feeds the Gram; G is
    diagonally dominated, measured end-to-end rel err ~6e-3) and rides the
    Pool SWDGE path whose descriptor prep overlaps the first transfer.
  - folded weights are ~160KB (vs 262KB for the P/Q form) on the two
    activation-queue HWDGE slots, ordered Wk|Wvff|identity first.
  - a chain of tiny matmuls warms the PE p-state ramp before the real work.
  - X^T comes from 8 PE transposes placed in PE idle gaps; the PSUM->SBUF
    staging copies run on the Activation engine, off the critical chain.
  - both result halves leave as one HWDGE store (a second store's 625ns
    issue would serialize behind the first and cost more than the extra
    273ns of transfer).
"""

import numpy as np
from contextlib import ExitStack

import ml_dtypes
import concourse.bass as bass
import concourse.tile as tile
from concourse import bacc, mybir
from concourse import bass_utils

B, N, E = 4, 2048, 96
H = 6
D = E // H            # 16
EA = E + 1            # 97 (augmented ones column)
NH = N // 2           # 1024 rows per half
NCH = 8               # row chunks per half (chunk j = rows {8p + j})
SCALE = float(D) ** -0.5
F32 = mybir.dt.float32
F16 = mybir.dt.float16
F8 = mybir.dt.float8e4
NP_F8 = ml_dtypes.float8_e4m3

N_WARM = 12           # PE p-state warmup matmuls
WARM_COLS = 128

# wpack (fp16, 97 partitions) column layout: Wk_aug | Wvff | identity
C_WK = 0              # Wk_aug [97, 96]
C_WVFF = 96           # Wvff (6 heads x [97, 96], scale folded)
C_ID = 672            # [97, 128] f16 identity (PE transpose operand)
WCOLS = 800
# wq2 (fp16, 16 partitions): per-head Wq_aug_h^T [16, 97] | onehot | bff
C_OH = 582            # [1, 97] onehot row (1.0 at col 96) - bff placement
C_BF = 679            # [1, 96] bff row
WQCOLS = 775

N_CORES = 8

_NC_CACHE = {}
LAST_RESULTS = None


def _build_nc():
    nc = bacc.Bacc(
        "TRN2", target_bir_lowering=False, debug=False, num_devices=N_CORES,
    )
    xa = nc.dram_tensor("xa", [NH, EA], F16, kind="ExternalInput").ap()
    xb = nc.dram_tensor("xb", [NH, EA], F8, kind="ExternalInput").ap()
    wpi = nc.dram_tensor("wpack", [128, WCOLS], F16, kind="ExternalInput").ap()
    wqi = nc.dram_tensor("wq2", [D, WQCOLS], F16, kind="ExternalInput").ap()
    outd = nc.dram_tensor("out", [128, 8 * E], F16, kind="ExternalOutput").ap()

    with tile.TileContext(nc) as tc, ExitStack() as ctx:
        consts = ctx.enter_context(tc.tile_pool(name="consts", bufs=1))
        big = ctx.enter_context(tc.tile_pool(name="big", bufs=1))
        small = ctx.enter_context(tc.tile_pool(name="small", bufs=1))
        outp = ctx.enter_context(tc.tile_pool(name="outp", bufs=1))
        ps_gw = ctx.enter_context(tc.tile_pool(name="ps_gw", bufs=1, space="PSUM"))
        ps_v = ctx.enter_context(tc.tile_pool(name="ps_v", bufs=1, space="PSUM"))
        ps_c = ctx.enter_context(tc.tile_pool(name="ps_c", bufs=2, space="PSUM"))
        ps_t = ctx.enter_context(tc.tile_pool(name="ps_t", bufs=2, space="PSUM"))
        ps_o = ctx.enter_context(tc.tile_pool(name="ps_o", bufs=2, space="PSUM"))

        # --- near half fp16 on the first sync-HWDGE slot; far half fp8 on
        # the Pool SWDGE path (its prep overlaps xa's transfer); weights on
        # the activation HWDGE queue, Wk|Wvff|identity first
        XA = big.tile([128, NCH, EA], F16)
        nc.sync.dma_start(out=XA[:], in_=xa.rearrange("(p j) e -> p j e", j=NCH))
        XB = big.tile([128, NCH, EA], F8)
        nc.gpsimd.dma_start(out=XB[:], in_=xb.rearrange("(p j) e -> p j e", j=NCH))
        wp = consts.tile([128, WCOLS], F16)
        nc.scalar.dma_start(out=wp[:], in_=wpi)
        wq2 = consts.tile([D, WQCOLS], F16)
        nc.scalar.dma_start(out=wq2[:], in_=wqi)

        # --- PE p-state warmup: keep the tensor engine busy from ~0.7us so
        # the ramp model is past the slow state when the real matmuls start
        wu = small.tile([1, WARM_COLS], F16)
        nc.vector.memset(wu[:], 0.0)
        wu_ps = ps_o.tile([1, WARM_COLS], F32, tag="og", name="warm")
        for _ in range(N_WARM):
            nc.tensor.matmul(
                wu_ps[:], lhsT=wu[0:1, 0:1], rhs=wu[:], start=True, stop=True
            )

        # --- G = X_aug^T X_aug, one 16-matmul PSUM accumulation group
        # (near half first - it arrives first)
        g_ps = ps_gw.tile([EA, EA], F32, tag="gw", name="g")
        for c in range(NCH):
            xc = XA[:, c, :]
            nc.tensor.matmul(g_ps[:], lhsT=xc, rhs=xc, start=(c == 0), stop=False)
        for c in range(NCH):
            xc = XB[:, c, :]
            nc.tensor.matmul(
                g_ps[:], lhsT=xc, rhs=xc, start=False, stop=(c == NCH - 1)
            )
        g_h = small.tile([EA, EA], F16)
        nc.vector.tensor_copy(out=g_h[:], in_=g_ps[:])

        # --- 8 PE transposes of the near half (PE is idle while the chain
        # copies run); PSUM->SBUF staging on Act, off the critical chain
        XT = big.tile([EA, NCH, 128], F16)
        pts = []
        for grp in range(2):
            pt = ps_t.tile([EA, 4, 128], F16, tag="pt", name=f"pt{grp}")
            for j in range(4):
                nc.tensor.transpose(
                    out=pt[:, j, :], in_=XA[:, 4 * grp + j, :],
                    identity=wp[:, C_ID : C_ID + 128],
                )
            pts.append(pt)

        # --- V = G @ Wk_aug
        v_ps = ps_v.tile([EA, E], F32)
        nc.tensor.matmul(
            v_ps[:], lhsT=g_h[:], rhs=wp[0:EA, C_WK : C_WK + E], start=True, stop=True
        )
        v_h = small.tile([EA, E], F16)
        nc.vector.tensor_copy(out=v_h[:], in_=v_ps[:])
        for grp in range(2):
            nc.scalar.copy(
                out=XT[:, 4 * grp : 4 * (grp + 1), :], in_=pts[grp][:]
            )

        # --- Call[0:16, 96h:96h+96] = V_h^T Wvff_h (PSUM base-partition rule
        # forces head blocks onto the free dim; two banks, DVE + Act copies)
        call_sb = small.tile([D, H * E], F16)
        for half in range(2):
            ca_ps = ps_c.tile([D, 3 * E], F32, tag="call", name=f"call{half}")
            for hh in range(3):
                h = 3 * half + hh
                nc.tensor.matmul(
                    ca_ps[:, E * hh : E * (hh + 1)],
                    lhsT=v_h[:, D * h : D * (h + 1)],
                    rhs=wp[0:EA, C_WVFF + E * h : C_WVFF + E * (h + 1)],
                    start=True, stop=True,
                )
            cp = nc.vector.tensor_copy if half == 0 else nc.scalar.copy
            cp(out=call_sb[:, 3 * E * half : 3 * E * (half + 1)], in_=ca_ps[:])

        # --- Wfin = sum_h Wq_aug_h @ Call_h + e_last bff^T (one accum group,
        # PSUM bank shared with G - dead after g_h)
        wf_ps = ps_gw.tile([EA, E], F32, tag="gw", name="wf")
        for h in range(H):
            nc.tensor.matmul(
                wf_ps[:],
                lhsT=wq2[:, EA * h : EA * (h + 1)],
                rhs=call_sb[:, E * h : E * (h + 1)],
                start=(h == 0), stop=False,
            )
        nc.tensor.matmul(
            wf_ps[:],
            lhsT=wq2[0:1, C_OH : C_OH + EA],
            rhs=wq2[0:1, C_BF : C_BF + E],
            start=False, stop=True,
        )
        wf_h = small.tile([EA, E], F16)
        nc.vector.tensor_copy(out=wf_h[:], in_=wf_ps[:])

        # --- finals: out rows {8p+j} = X_chunk @ Wfin; each half leaves as
        # its own HWDGE store so the issues/transfers overlap
        osb = outp.tile([128, 2, 4 * E], F16)
        for g in range(2):
            og = ps_o.tile([128, 4, E], F32, tag="og", name=f"og{g}")
            for j4 in range(4):
                nc.tensor.matmul(
                    og[:, j4, :], lhsT=XT[:, 4 * g + j4, :], rhs=wf_h[:],
                    start=True, stop=True,
                )
            cp = nc.vector.tensor_copy if g == 0 else nc.scalar.copy
            cp(out=osb[:, g, :], in_=og[:].rearrange("p a b -> p (a b)"))
        nc.sync.dma_start(out=outd, in_=osb[:].rearrange("p a b -> p (a b)"))

    nc.compile()
    return nc


def get_nc():
    if "nc" not in _NC_CACHE:
        _NC_CACHE["nc"] = _build_nc()
    return _NC_CACHE["nc"]


def _host_weights(Wqkv, bqkv, Wff, bff):
    waug = np.concatenate(
        [np.asarray(Wqkv, np.float64), np.asarray(bqkv, np.float64)[None, :]], axis=0
    )
    Wq, Wk, Wv = waug[:, 0:E], waug[:, E : 2 * E], waug[:, 2 * E : 3 * E]
    Wff = np.asarray(Wff, np.float64)
    wp = np.zeros((128, WCOLS), np.float16)
    wp[0:EA, C_WK : C_WK + E] = Wk.astype(np.float16)
    wp[:, C_ID : C_ID + 128] = np.eye(128, dtype=np.float16)
    wq2 = np.zeros((D, WQCOLS), np.float16)
    for h in range(H):
        hd = slice(h * D, (h + 1) * D)
        wp[0:EA, C_WVFF + E * h : C_WVFF + E * (h + 1)] = (
            SCALE * (Wv[:, hd] @ Wff[hd, :])
        ).astype(np.float16)
        wq2[:, EA * h : EA * (h + 1)] = Wq[:, hd].T.astype(np.float16)
    wq2[0, C_OH + E] = 1.0
    wq2[0, C_BF : C_BF + E] = np.asarray(bff, np.float16)
    return {"wpack": wp, "wq2": wq2}


def make_in_maps(x, Wqkv, bqkv, Wff, bff):
    x = np.asarray(x, np.float32)
    w = _host_weights(Wqkv, bqkv, Wff, bff)
    x16 = x.astype(np.float16)
    in_maps = []
    for c in range(N_CORES):
        b, h = divmod(c, 2)
        mine = x16[b, h * NH : (h + 1) * NH]
        other = x16[b, (1 - h) * NH : (2 - h) * NH]
        xa = np.ones((NH, EA), np.float16)
        xa[:, 0:E] = mine
        xbm = np.ones((NH, EA), np.float16)
        xbm[:, 0:E] = other
        m = {"xa": xa, "xb": xbm.astype(NP_F8)}
        m.update(w)
        in_maps.append(m)
    return in_maps


def assemble(results):
    out = np.empty((B, N, E), np.float32)
    for c in range(N_CORES):
        b, h = divmod(c, 2)
        half = results[c]["out"].reshape(128, 8, E).astype(np.float32)
        out[b, h * NH : (h + 1) * NH] = half.reshape(NH, E)
    return out


def kernel(x, Wqkv, bqkv, Wff, bff):
    global LAST_RESULTS
    nc = get_nc()
    in_maps = make_in_maps(x, Wqkv, bqkv, Wff, bff)
    res = bass_utils.run_bass_kernel_spmd(
        nc, in_maps, core_ids=list(range(N_CORES))
    )
    LAST_RESULTS = res
    return assemble(res.results)


# revision 43
# speedup vs baseline: 1.0094x; 1.0094x over previous
"""Trainium2 Bass kernel for nn_MultiHeadAttention_7584912245188.

Reference computes (no softmax!):
    qkv = x @ Wqkv + bqkv ; split q,k,v ; per head: y = (q k^T / sqrt(D)) v
    out = y @ Wff + bff

No softmax => attention is linear and reassociates: (Q K^T) V = Q (K^T V).
With X_aug = [X | 1] ([N, 97]) and G = X_aug^T X_aug ([97, 97]) the module
collapses to out = X_aug @ Wfin computed on device as:
    V = G @ Wk_aug                                  [97, 96]  (1 matmul)
    Call_h = V_h^T @ Wvff_h                         [16, 96]  (6 matmuls,
             head blocks along the free dim; Wvff_h = D^-0.5 Wv_aug_h Wff_h)
    Wfin = sum_h Wq_aug_h @ Call_h + e_last bff^T   [97, 96]  (7-matmul group)
    out rows {8p+j} = X_chunk @ Wfin                (8 matmuls via transposed X)
O(N*E^2) instead of O(N^2*D).

Sharding (8 cores): core c -> (batch b = c//2, half h = c%2). Each core
computes the full-batch Gram redundantly (cheaper than a collective) and
writes its own half of the rows.

Schedule (fixed DMA costs dominate: HWDGE issue ~630 + 650 DGE delay, SWDGE
prep ~1040 + 650, 900ns completion sem, shared ~360GB/s DMA engines):
  - near half rides the first sync-HWDGE slot in fp16 (Gram + transposes);
    the far half is QUANTIZED TO FP8 (it only feeds the Gram; G is
    diagonally dominated, measured end-to-end rel err ~6e-3) and rides the
    Pool SWDGE path whose descriptor prep overlaps the first transfer.
  - folded weights are ~160KB (vs 262KB for the P/Q form) on the two
    activation-queue HWDGE slots, ordered Wk|Wvff|identity first.
  - a chain of tiny matmuls warms the PE p-state ramp before the real work.
  - X^T comes from 8 PE transposes placed in PE idle gaps; the PSUM->SBUF
    staging copies run on the Activation engine, off the critical chain.
  - both result halves leave as one HWDGE store (a second store's 625ns
    issue would serialize behind the first and cost more than the extra
    273ns of transfer).
"""

import numpy as np
from contextlib import ExitStack

import ml_dtypes
import concourse.bass as bass
import concourse.tile as tile
from concourse import bacc, mybir
from concourse import bass_utils

B, N, E = 4, 2048, 96
H = 6
D = E // H            # 16
EA = E + 1            # 97 (augmented ones column)
NH = N // 2           # 1024 rows per half
NCH = 8               # row chunks per half (chunk j = rows {8p + j})
SCALE = float(D) ** -0.5
F32 = mybir.dt.float32
F16 = mybir.dt.float16
F8 = mybir.dt.float8e4
NP_F8 = ml_dtypes.float8_e4m3

N_WARM = 12           # PE p-state warmup matmuls
WARM_COLS = 128

# wpack (fp16, 97 partitions) column layout: Wk_aug | Wvff | identity
C_WK = 0              # Wk_aug [97, 96]
C_WVFF = 96           # Wvff (6 heads x [97, 96], scale folded)
C_ID = 672            # [97, 128] f16 identity (PE transpose operand)
WCOLS = 800
# wq2 (fp16, 16 partitions): per-head Wq_aug_h^T [16, 97] | onehot | bff
C_OH = 582            # [1, 97] onehot row (1.0 at col 96) - bff placement
C_BF = 679            # [1, 96] bff row
WQCOLS = 775

N_CORES = 8

_NC_CACHE = {}
LAST_RESULTS = None


def _build_nc():
    nc = bacc.Bacc(
        "TRN2", target_bir_lowering=False, debug=False, num_devices=N_CORES,
    )
    xa = nc.dram_tensor("xa", [NH, EA], F16, kind="ExternalInput").ap()
    xb = nc.dram_tensor("xb", [NH, EA], F8, kind="ExternalInput").ap()
    wpi = nc.dram_tensor("wpack", [128, WCOLS], F16, kind="ExternalInput").ap()
    wqi = nc.dram_tensor("wq2", [D, WQCOLS], F16, kind="ExternalInput").ap()
    outd = nc.dram_tensor("out", [128, 8 * E], F16, kind="ExternalOutput").ap()

    with tile.TileContext(nc) as tc, ExitStack() as ctx:
        consts = ctx.enter_context(tc.tile_pool(name="consts", bufs=1))
        big = ctx.enter_context(tc.tile_pool(name="big", bufs=1))
        small = ctx.enter_context(tc.tile_pool(name="small", bufs=1))
        outp = ctx.enter_context(tc.tile_pool(name="outp", bufs=1))
        ps_gw = ctx.enter_context(tc.tile_pool(name="ps_gw", bufs=1, space="PSUM"))
        ps_v = ctx.enter_context(tc.tile_pool(name="ps_v", bufs=1, space="PSUM"))
        ps_c = ctx.enter_context(tc.tile_pool(name="ps_c", bufs=2, space="PSUM"))
        ps_t = ctx.enter_context(tc.tile_pool(name="ps_t", bufs=2, space="PSUM"))
        ps_o = ctx.enter_context(tc.tile_pool(name="ps_o", bufs=2, space="PSUM"))

        # --- near half fp16 on the first sync-HWDGE slot; far half fp8 on
        # the Pool SWDGE path (its prep overlaps xa's transfer); weights on
        # the activation HWDGE queue, Wk|Wvff|identity first
        XA = big.tile([128, NCH, EA], F16)
        nc.sync.dma_start(out=XA[:], in_=xa.rearrange("(p j) e -> p j e", j=NCH))
        XB = big.tile([128, NCH, EA], F8)
        nc.gpsimd.dma_start(out=XB[:], in_=xb.rearrange("(p j) e -> p j e", j=NCH))
        wp = consts.tile([128, WCOLS], F16)
        nc.scalar.dma_start(out=wp[:], in_=wpi)
        wq2 = consts.tile([D, WQCOLS], F16)
        nc.scalar.dma_start(out=wq2[:], in_=wqi)

        # --- PE p-state warmup: keep the tensor engine busy from ~0.7us so
        # the ramp model is past the slow state when the real matmuls start
        wu = small.tile([1, WARM_COLS], F16)
        nc.vector.memset(wu[:], 0.0)
        wu_ps = ps_o.tile([1, WARM_COLS], F32, tag="og", name="warm")
        for _ in range(N_WARM):
            nc.tensor.matmul(
                wu_ps[:], lhsT=wu[0:1, 0:1], rhs=wu[:], start=True, stop=True
            )

        # --- G = X_aug^T X_aug, one 16-matmul PSUM accumulation group
        # (near half first - it arrives first)
        g_ps = ps_gw.tile([EA, EA], F32, tag="gw", name="g")
        for c in range(NCH):
            xc = XA[:, c, :]
            nc.tensor.matmul(g_ps[:], lhsT=xc, rhs=xc, start=(c == 0), stop=False)
        for c in range(NCH):
            xc = XB[:, c, :]
            nc.tensor.matmul(
                g_ps[:], lhsT=xc, rhs=xc, start=False, stop=(c == NCH - 1)
            )
        g_h = small.tile([EA, EA], F16)
        nc.vector.tensor_copy(out=g_h[:], in_=g_ps[:])

        # --- 8 PE transposes of the near half (PE is idle while the chain
        # copies run); PSUM->SBUF staging on Act, off the critical chain
        XT = big.tile([EA, NCH, 128], F16)
        pts = []
        for grp in range(2):
            pt = ps_t.tile([EA, 4, 128], F16, tag="pt", name=f"pt{grp}")
            for j in range(4):
                nc.tensor.transpose(
                    out=pt[:, j, :], in_=XA[:, 4 * grp + j, :],
                    identity=wp[:, C_ID : C_ID + 128],
                )
            pts.append(pt)

        # --- V = G @ Wk_aug
        v_ps = ps_v.tile([EA, E], F32)
        nc.tensor.matmul(
            v_ps[:], lhsT=g_h[:], rhs=wp[0:EA, C_WK : C_WK + E], start=True, stop=True
        )
        v_h = small.tile([EA, E], F16)
        nc.vector.tensor_copy(out=v_h[:], in_=v_ps[:])
        nc.scalar.copy(out=XT[:, 0:4, :], in_=pts[0][:])

        # --- Call[0:16, 96h:96h+96] = V_h^T Wvff_h (PSUM base-partition rule
        # forces head blocks onto the free dim; two banks, DVE + Act copies)
        call_sb = small.tile([D, H * E], F16)
        for half in range(2):
            ca_ps = ps_c.tile([D, 3 * E], F32, tag="call", name=f"call{half}")
            for hh in range(3):
                h = 3 * half + hh
                nc.tensor.matmul(
                    ca_ps[:, E * hh : E * (hh + 1)],
                    lhsT=v_h[:, D * h : D * (h + 1)],
                    rhs=wp[0:EA, C_WVFF + E * h : C_WVFF + E * (h + 1)],
                    start=True, stop=True,
                )
            cp = nc.vector.tensor_copy if half == 0 else nc.scalar.copy
            cp(out=call_sb[:, 3 * E * half : 3 * E * (half + 1)], in_=ca_ps[:])
        # XT second half staged after the Call copy - it is only needed by
        # the last four finals, and this keeps Act off the critical chain
        nc.scalar.copy(out=XT[:, 4:8, :], in_=pts[1][:])

        # --- Wfin = sum_h Wq_aug_h @ Call_h + e_last bff^T (one accum group,
        # PSUM bank shared with G - dead after g_h)
        wf_ps = ps_gw.tile([EA, E], F32, tag="gw", name="wf")
        for h in range(H):
            nc.tensor.matmul(
                wf_ps[:],
                lhsT=wq2[:, EA * h : EA * (h + 1)],
                rhs=call_sb[:, E * h : E * (h + 1)],
                start=(h == 0), stop=False,
            )
        nc.tensor.matmul(
            wf_ps[:],
            lhsT=wq2[0:1, C_OH : C_OH + EA],
            rhs=wq2[0:1, C_BF : C_BF + E],
            start=False, stop=True,
        )
        wf_h = small.tile([EA, E], F16)
        nc.vector.tensor_copy(out=wf_h[:], in_=wf_ps[:])

        # --- finals: out rows {8p+j} = X_chunk @ Wfin; each half leaves as
        # its own HWDGE store so the issues/transfers overlap
        osb = outp.tile([128, 2, 4 * E], F16)
        for g in range(2):
            og = ps_o.tile([128, 4, E], F32, tag="og", name=f"og{g}")
            for j4 in range(4):
                nc.tensor.matmul(
                    og[:, j4, :], lhsT=XT[:, 4 * g + j4, :], rhs=wf_h[:],
                    start=True, stop=True,
                )
            cp = nc.vector.tensor_copy if g == 0 else nc.scalar.copy
            cp(out=osb[:, g, :], in_=og[:].rearrange("p a b -> p (a b)"))
        nc.sync.dma_start(out=outd, in_=osb[:].rearrange("p a b -> p (a b)"))

    nc.compile()
    return nc


def get_nc():
    if "nc" not in _NC_CACHE:
        _NC_CACHE["nc"] = _build_nc()
    return _NC_CACHE["nc"]


def _host_weights(Wqkv, bqkv, Wff, bff):
    waug = np.concatenate(
        [np.asarray(Wqkv, np.float64), np.asarray(bqkv, np.float64)[None, :]], axis=0
    )
    Wq, Wk, Wv = waug[:, 0:E], waug[:, E : 2 * E], waug[:, 2 * E : 3 * E]
    Wff = np.asarray(Wff, np.float64)
    wp = np.zeros((128, WCOLS), np.float16)
    wp[0:EA, C_WK : C_WK + E] = Wk.astype(np.float16)
    wp[:, C_ID : C_ID + 128] = np.eye(128, dtype=np.float16)
    wq2 = np.zeros((D, WQCOLS), np.float16)
    for h in range(H):
        hd = slice(h * D, (h + 1) * D)
        wp[0:EA, C_WVFF + E * h : C_WVFF + E * (h + 1)] = (
            SCALE * (Wv[:, hd] @ Wff[hd, :])
        ).astype(np.float16)
        wq2[:, EA * h : EA * (h + 1)] = Wq[:, hd].T.astype(np.float16)
    wq2[0, C_OH + E] = 1.0
    wq2[0, C_BF : C_BF + E] = np.asarray(bff, np.float16)
    return {"wpack": wp, "wq2": wq2}


def make_in_maps(x, Wqkv, bqkv, Wff, bff):
    x = np.asarray(x, np.float32)
    w = _host_weights(Wqkv, bqkv, Wff, bff)
    x16 = x.astype(np.float16)
    in_maps = []
    for c in range(N_CORES):
        b, h = divmod(c, 2)
        mine = x16[b, h * NH : (h + 1) * NH]
        other = x16[b, (1 - h) * NH : (2 - h) * NH]
        xa = np.ones((NH, EA), np.float16)
        xa[:, 0:E] = mine
        xbm = np.ones((NH, EA), np.float16)
        xbm[:, 0:E] = other
        m = {"xa": xa, "xb": xbm.astype(NP_F8)}
        m.update(w)
        in_maps.append(m)
    return in_maps


def assemble(results):
    out = np.empty((B, N, E), np.float32)
    for c in range(N_CORES):
        b, h = divmod(c, 2)
        half = results[c]["out"].reshape(128, 8, E).astype(np.float32)
        out[b, h * NH : (h + 1) * NH] = half.reshape(NH, E)
    return out


def kernel(x, Wqkv, bqkv, Wff, bff):
    global LAST_RESULTS
    nc = get_nc()
    in_maps = make_in_maps(x, Wqkv, bqkv, Wff, bff)
    res = bass_utils.run_bass_kernel_spmd(
        nc, in_maps, core_ids=list(range(N_CORES))
    )
    LAST_RESULTS = res
    return assemble(res.results)
